# revision 119
# baseline (speedup 1.0000x reference)
"""GAT layer (8-head attention + 4-hop propagation + LayerNorm) on 8 TRN2 cores.

Sharding: data-parallel over batch B=8 — core b computes batch b entirely
(no collectives). Per-core math (projections/scores in bf16, E/z bf16,
fp32 psum accumulation; measured rel err 5.3e-3 vs f64 reference):

  qT/kT = Wq/Wk @ x^T + b      [512, 1024]  (hidden on partitions)
  v01   = 0.1*(x @ Wv^T + bv)  [1024, 512]  (nodes on partitions)
  per head h (64-dim slices of qT/kT/v01):
    E[m, n] = exp(k_h[m]·q_h[n]) * mask[m, n]          (scores transposed)
    D[n]    = sum_m E[m, n]      (ones-column matmul, fused with hop 1)
    z1 = (E.T @ v01_h) * (9/D)  + v01_h                (= 0.9*P@v + 0.1*v)
    z_{t+1} = (E.T @ z_t) * (0.9/D) + v01_h            (hops 2..4)
    y[:, h*64:+64] = z4 + x_head  (residual folded into hop 3's add)
  out = LayerNorm(y) -> bf16 -> host casts back to fp32

The softmax max-subtraction is skipped: scores are bounded (|S| < 25) so
exp() cannot overflow fp32, and exp of the masked scores times the 0/1
mask exactly matches the reference's masked softmax.

Scheduling (cost-model-guided; modeled ~126us/core vs 287us baseline):
  - all big matmuls bf16 (1 PE cycle/row vs 4 for fp32; FWL weight loads)
  - software pipelining: head h+1's scores/exp are interleaved into head
    h's hop loop 2 mt-tiles per hop, pacing PE issue to scps psum frees so
    the engine FIFOs never head-block on ACT's exp chain
  - elementwise spread across DVE and Pool (mask mults alternate, z-adds
    and LN tensor_scalar on Pool, per-head bn_stats in DVE slack windows)
  - hop psum in two half-tiles so each half's scale-mult starts at its
    own accumulation stop, shortening the z-chain between hops
  - first DMA chunks sized small (256-col) to beat the ~1.7us DGE landing
    latency; mask/x/w loads spread over the SP/ACT/Pool queues
"""

import numpy as np

import concourse.bass as bass
import concourse.mybir as mybir
import concourse.tile as tile
from concourse import bacc
from concourse.bass_utils import run_bass_kernel_spmd

B, N, H = 8, 1024, 512
NHEADS, U = 8, 64
P = 128
NT = N // P          # 8 node tiles
KT = H // P          # 4 hidden tiles
ALPHA = 0.1
LN_EPS = 1e-5
F32 = mybir.dt.float32
BF16 = mybir.dt.bfloat16

# E/z dtype for the propagation path: "float32" (rel err ~6e-6) or
# "bfloat16" (rel err ~1.2e-3, ~2x faster weight loads on the hop matmuls).
E_DTYPE_NAME = "bfloat16"

_BUILD_CACHE = {}


def build_nc(edt_name: str, apply_affine: bool, out_bf16: bool = True):
    key = (edt_name, apply_affine, out_bf16)
    if key in _BUILD_CACHE:
        return _BUILD_CACHE[key]

    EDT = getattr(mybir.dt, edt_name)
    ZDT = EDT
    ODT = BF16 if out_bf16 else F32
    nc = bacc.Bacc(None, target_bir_lowering=False)

    xT_d = nc.dram_tensor("xT", [H, N], BF16, kind="ExternalInput")
    xn_d = nc.dram_tensor("xn", [N, H], F32, kind="ExternalInput")
    maskT_d = nc.dram_tensor("maskT", [N, N], EDT, kind="ExternalInput")
    wq_d = nc.dram_tensor("wqT", [H, H], BF16, kind="ExternalInput")
    wk_d = nc.dram_tensor("wkT", [H, H], BF16, kind="ExternalInput")
    wv_d = nc.dram_tensor("wvT01", [H, H], BF16, kind="ExternalInput")
    bq_d = nc.dram_tensor("bq", [H], F32, kind="ExternalInput")
    bk_d = nc.dram_tensor("bk", [H], F32, kind="ExternalInput")
    bv_d = nc.dram_tensor("bv01", [H], BF16, kind="ExternalInput")
    if apply_affine:
        gam_d = nc.dram_tensor("gammar", [P, H], F32, kind="ExternalInput")
        bet_d = nc.dram_tensor("betar", [P, H], F32, kind="ExternalInput")
    # out in bf16 by default (host converts back): halves the tail DMA
    out_d = nc.dram_tensor("out", [N, H], ODT, kind="ExternalOutput")

    with tile.TileContext(nc) as tc:
        with tc.tile_pool(name="const", bufs=1) as cpool, \
             tc.tile_pool(name="big", bufs=1) as bpool, \
             tc.tile_pool(name="epool", bufs=3 if edt_name == "bfloat16" else 1) as epool, \
             tc.tile_pool(name="zpool", bufs=3) as zpool, \
             tc.tile_pool(name="tpool", bufs=2) as tpool, \
             tc.tile_pool(name="spool", bufs=8) as spool, \
             tc.tile_pool(name="wxpool", bufs=2) as wxpool, \
             tc.tile_pool(name="ph1", bufs=1) as p1, \
             tc.tile_pool(name="scps", bufs=3, space="PSUM") as scps, \
             tc.tile_pool(name="dps", bufs=2, space="PSUM") as dpsp:

            # ---- persistent SBUF residents ----
            # Loads spread over the sync/gpsimd/scalar/vector DMA queues so
            # the PE-critical tensors (xT + weights) land first in parallel.
            bq_sb = cpool.tile([P, KT], F32)
            bk_sb = cpool.tile([P, KT], F32)
            bvrow_sb = cpool.tile([P, H], BF16, tag="bvrow")
            maskT_sb = cpool.tile([P, NT, N], EDT)
            xn_sb = cpool.tile([P, NT, H], F32)
            ones_row = cpool.tile([P, P], BF16, tag="onesrow")
            nc.vector.memset(ones_row[:1, :], 1.0)
            eps_sb = cpool.tile([P, 1], F32, tag="eps")
            nc.vector.memset(eps_sb[:], LN_EPS)
            if apply_affine:
                gam_sb = cpool.tile([P, H], F32, tag="gam")
                bet_sb = cpool.tile([P, H], F32, tag="bet")

            qT_sb = bpool.tile([P, KT, N], BF16, tag="qT")
            kT_sb = bpool.tile([P, KT, N], BF16, tag="kT")
            # v01 per-head blocks of 64 values + a trailing 1.0 column; the
            # ones column rides hop 1's moving operand to produce D in PSUM.
            v01_sb = bpool.tile([P, NT, NHEADS, U + 1], ZDT, tag="v01")
            nc.vector.memset(v01_sb[:, :, :, U:U + 1], 1.0)
            y_sb = bpool.tile([P, NT, H], F32, tag="y")
            ybf_sb = bpool.tile([P, NT, H], ODT, tag="ybf")
            # LN partial stats per head: head h's stats are interleaved into
            # head h+1's hop loop (2 per hop, in DVE slack); head 7's run in
            # the tail. (walrus: bn_stats out must be exactly [P, 6].)
            st6_sb = bpool.tile([P, NT, NHEADS, 6], F32, tag="st6all")

            # ---- phase 1: projections ----
            if True:
                # xT split into two TILES on two queues: separate tiles keep
                # the first q matmuls off the second half's completion.
                xT_r = xT_d[:, :].rearrange("(t p) n -> p t n", p=P)
                wq_r = wq_d[:, :].rearrange("(t p) i -> p t i", p=P)
                # first chunks small: DMA landing = transfer end + ~1.7us
                # DGE latency, so a 256-col xT chunk + the it0/1 wq columns
                # unblock the first matmul group ~0.9us earlier
                xTa0_sb = p1.tile([P, KT, 256], BF16, tag="xTa0")
                xTa1_sb = p1.tile([P, KT, 256], BF16, tag="xTa1")
                xTb_sb = p1.tile([P, KT, 512], BF16, tag="xTb")
                wq01_sb = p1.tile([P, KT, 256], BF16, tag="wq01")
                wq23_sb = p1.tile([P, KT, 256], BF16, tag="wq23")
                nc.sync.dma_start(xTa0_sb[:], xT_r[:, :, 0:256])
                nc.sync.dma_start(xTa1_sb[:], xT_r[:, :, 256:512])
                nc.sync.dma_start(xTb_sb[:], xT_r[:, :, 512:N])
                nc.gpsimd.dma_start(wq01_sb[:], wq_r[:, :, 0:256])
                nc.gpsimd.dma_start(wq23_sb[:], wq_r[:, :, 256:512])
                wk_sb = p1.tile([P, KT, H], BF16, tag="wk")
                nc.scalar.dma_start(wk_sb[:], wk_d[:, :].rearrange("(t p) i -> p t i", p=P))
                wv_sb = p1.tile([P, KT, H], BF16, tag="wv")
                nc.scalar.dma_start(wv_sb[:], wv_d[:, :].rearrange("(t p) i -> p t i", p=P))
                nc.gpsimd.dma_start(bq_sb[:], bq_d[:].rearrange("(t p) -> p t", p=P))
                nc.gpsimd.dma_start(bk_sb[:], bk_d[:].rearrange("(t p) -> p t", p=P))
                nc.gpsimd.dma_start(bvrow_sb[:1, :],
                                    bv_d[:].rearrange("(a h) -> a h", a=1))
                # mask halves avoid the ACT queue so exp(h0) isn't stuck
                # behind a DMA transfer; ACT queue stays clear after wk/wv.
                maskT_r = maskT_d[:, :].rearrange("(t p) n -> p t n", p=P)
                nc.gpsimd.dma_start(maskT_sb[:, 0:4, :], maskT_r[:, 0:4, :])
                nc.sync.dma_start(maskT_sb[:, 4:8, :], maskT_r[:, 4:8, :])
                nc.gpsimd.dma_start(xn_sb[:], xn_d[:, :].rearrange("(t p) h -> p t h", p=P))
                if apply_affine:
                    nc.gpsimd.dma_start(gam_sb[:], gam_d[:, :])
                    nc.gpsimd.dma_start(bet_sb[:], bet_d[:, :])

                # qT[i, n] = sum_k WqT[k, i] xT[k, n] + bq[i]
                # bf16 matmuls: 1 cyc/row + FWL weight loads on HW.
                # q weights come as two column-halves; the moving side of
                # ncx=0 as two 256-col chunks (separate accumulation groups
                # in one psum tile, drained by one bias add).
                def wq_slice(it):
                    w = wq01_sb if it < 2 else wq23_sb
                    return w[:, :, (it % 2) * P:(it % 2 + 1) * P]

                qk_x_chunks = [
                    [(xTa0_sb, 0, 256), (xTa1_sb, 256, 256)],
                    [(xTb_sb, 0, 512)],
                ]

                def qk_proj(it):
                    for wsel, b_sb, dst in ((wq_slice, bq_sb, qT_sb),
                                            (None, bk_sb, kT_sb)):
                        w_it = (wsel(it) if wsel
                                else wk_sb[:, :, it * P:(it + 1) * P])
                        for ncx in range(2):
                            ps = dpsp.tile([P, 512], F32, tag="hps1")
                            for x_sb, c0, cw in qk_x_chunks[ncx]:
                                for kt in range(KT):
                                    nc.tensor.matmul(
                                        ps[:, c0:c0 + cw],
                                        w_it[:, kt, :],
                                        x_sb[:, kt, :],
                                        start=(kt == 0), stop=(kt == KT - 1),
                                    )
                            nc.vector.tensor_scalar_add(
                                dst[:, it, ncx * 512:(ncx + 1) * 512], ps[:],
                                b_sb[:, it:it + 1],
                            )

                # v01[node, j] = sum_k xT[k, node] WvT01[k, j] + bv01[j]
                def xt_slice(nt):
                    if nt < 2:
                        return xTa0_sb[:, :, nt * P:(nt + 1) * P]
                    if nt < 4:
                        return xTa1_sb[:, :, (nt - 2) * P:(nt - 1) * P]
                    return xTb_sb[:, :, (nt - 4) * P:(nt - 3) * P]

                def v_proj(interleave=()):
                    interleave = list(interleave)
                    for nt in range(NT):
                        if interleave:
                            interleave.pop(0)()
                        x_nt = xt_slice(nt)
                        ps = dpsp.tile([P, 512], F32, tag="hps1")
                        nc.tensor.matmul(
                            ps[:], ones_row[:1, :P],
                            bvrow_sb[:1, :],
                            start=True, stop=False,
                        )
                        for kt in range(KT):
                            nc.tensor.matmul(
                                ps[:],
                                x_nt[:, kt, :],
                                wv_sb[:, kt, :],
                                start=False, stop=(kt == KT - 1),
                            )
                        nc.vector.tensor_scalar_add(
                            v01_sb[:, nt, :, 0:U],
                            ps[:].rearrange("p (h u) -> p h u", u=U),
                            0.0,
                        )

            # ---- phase 2: per-head attention + propagation ----
            # Software-pipelined: head h+1's scores/exp/mask are ISSUED before
            # head h's hops, so the PE FIFO has ready matmuls (hops of h)
            # while ACT computes exp for h+1. Engine queues are in-order, so
            # program order is what creates cross-head overlap.
            def scores_exp_mask(h, defer=True, pool_masks=False):
                """Emit (or defer) head h's scores+exp+mask.

                Deferred form returns (e_sb, score_chunks, mask_thunks):
                score_chunks are 4 thunks of 2 mt-tiles each, to be popped
                one per hop of the PREVIOUS head's hop loop — interleaving
                them with ready hop matmuls keeps the PE FIFO from head-
                blocking on scps tiles that free at ACT's exp pace.
                """
                pt, po = h // 2, (h % 2) * U
                kh = kT_sb[po:po + U, pt, :]   # [64, 1024] (d on partitions)
                qh = qT_sb[po:po + U, pt, :]
                e_sb = epool.tile([P, NT, N], EDT, tag="E", name=f"E_{h}")
                masks = []

                def mask_op(mt):
                    # alternates DVE/Pool; mt6<->mt7 swapped so the last-
                    # produced tile (gates hop 0) takes DVE's shorter op;
                    # all-Pool when DVE is busy with projection drains (h=0)
                    on_pool = (mt % 2 == 1) if mt < 6 else (mt == 6)
                    eng = nc.gpsimd if (pool_masks or on_pool) else nc.vector
                    eng.tensor_tensor(
                        e_sb[:, mt, :], e_sb[:, mt, :], maskT_sb[:, mt, :],
                        mybir.AluOpType.mult,
                    )

                def score_mt(mt):
                    sps = scps.tile([P, N], F32, tag="scps")
                    for ncx in range(2):
                        nc.tensor.matmul(
                            sps[:, ncx * 512:(ncx + 1) * 512],
                            kh[:, mt * P:(mt + 1) * P],
                            qh[:, ncx * 512:(ncx + 1) * 512],
                            start=True, stop=True,
                        )
                    nc.scalar.activation(
                        e_sb[:, mt, :], sps[:], mybir.ActivationFunctionType.Exp,
                    )
                    if defer:
                        masks.append(lambda mt=mt: mask_op(mt))
                    else:
                        mask_op(mt)

                if not defer:
                    for mt in range(NT):
                        score_mt(mt)
                    return e_sb, None, None

                return e_sb, [lambda mt=mt: score_mt(mt) for mt in range(NT)], masks

            def hops(h, e_sb, next_scores=(), next_masks=()):
                # Previous head's LN stats (tiny [P,64] calls) and the NEXT
                # head's mask mults, deferred into this head's hop loop: a
                # few per hop land in DVE/Pool slack windows (a block of 8
                # at once would head-block the queues' hop-critical ops).
                stats_prev = []
                if h >= 1:
                    g = h - 1
                    stats_prev = [
                        lambda nt=nt, g=g: nc.vector.bn_stats(
                            st6_sb[:, nt, g, :],
                            y_sb[:, nt, g * U:(g + 1) * U])
                        for nt in range(NT)
                    ]
                # NOTE: next_masks is the NEXT head's live list — score
                # chunks popped below append to it as they run; don't copy.
                next_masks = next_masks if next_masks is not None else []
                next_scores = list(next_scores)
                w0 = v01_sb[:, :, h, 0:U]  # [128, 8, 64]
                # hop-3 combines t + (w0 + xn_head) in one add; precompute
                # the sum here so the tail chain is one Pool op shorter.
                w0xn = wxpool.tile([P, NT, U], F32, tag="w0xn", name=f"wx_{h}")
                nc.gpsimd.tensor_tensor(
                    w0xn[:], v01_sb[:, :, h, 0:U],
                    xn_sb[:, :, h * U:(h + 1) * U], mybir.AluOpType.add,
                )
                rd09 = spool.tile([P, NT], F32, tag="rd09")
                rd9 = spool.tile([P, NT], F32, tag="rd9")
                z_prev = None
                for hop in range(4):
                    # 2 mt-tiles of the next head's scores ahead of the
                    # hop's matmuls — keeps ACT's exp pipeline fed without
                    # head-blocking PE on scps frees
                    for _ in range(2):
                        if next_scores:
                            next_scores.pop(0)()
                    t = tpool.tile([P, NT, U], F32, tag="t")
                    if hop == 0:
                        # moving operand carries [z0 | 1]; D lands in col U.
                        # Two 1-bank psum tiles: a 65-col accumulation group
                        # cannot cross a PSUM bank boundary.
                        halves = [
                            dpsp.tile([P, NT // 2, U + 1], F32, tag="hps1",
                                      name=f"hps1_{h}_{i}")
                            for i in (0, 1)
                        ]
                        for nt in range(NT):
                            hp = halves[nt // 4]
                            for mt in range(NT):
                                nc.tensor.matmul(
                                    hp[:, nt % 4, :],
                                    e_sb[:, mt, nt * P:(nt + 1) * P],
                                    v01_sb[:, mt, h, :],
                                    start=(mt == 0), stop=(mt == NT - 1),
                                )
                        rdraw = spool.tile([P, NT], F32, tag="rdraw")
                        nc.vector.reciprocal(rdraw[:, 0:4], halves[0][:, :, U])
                        nc.vector.reciprocal(rdraw[:, 4:8], halves[1][:, :, U])
                        # rd9 first: it gates hop 0's t-mult; rd09 is only
                        # needed ~1.7us later by hop 1
                        nc.vector.tensor_scalar_mul(rd9[:], rdraw[:],
                                                    (1.0 - ALPHA) / ALPHA)
                        nc.vector.tensor_scalar_mul(rd09[:], rdraw[:], 1.0 - ALPHA)
                        for i in (0, 1):
                            nc.vector.tensor_tensor(
                                t[:, 4 * i:4 * (i + 1), :],
                                halves[i][:, :, 0:U],
                                rd9[:, 4 * i:4 * (i + 1), None].to_broadcast(
                                    [P, 4, U]),
                                mybir.AluOpType.mult,
                            )
                    else:
                        # psum as two half TILES: half A's mult fires at the
                        # nt=3 group's stop, overlapping half B's matmuls
                        hhs = [
                            dpsp.tile([P, NT // 2, U], F32, tag="hps1",
                                      name=f"hps_{h}_{hop}_{i}")
                            for i in (0, 1)
                        ]
                        for nt in range(NT):
                            for mt in range(NT):
                                nc.tensor.matmul(
                                    hhs[nt // 4][:, nt % 4, :],
                                    e_sb[:, mt, nt * P:(nt + 1) * P],
                                    z_prev[:, mt, :],
                                    start=(mt == 0), stop=(mt == NT - 1),
                                )
                        for i in (0, 1):
                            sl = slice(4 * i, 4 * (i + 1))
                            nc.vector.tensor_tensor(
                                t[:, sl, :], hhs[i][:],
                                rd09[:, sl, None].to_broadcast([P, 4, U]),
                                mybir.AluOpType.mult)
                    # SBUF-only adds -> Pool engine (DVE is near-saturated),
                    # in nt-halves so downstream consumers start earlier.
                    if hop == 3:
                        # y = t + (w0 + xn_head): residual folded via w0xn
                        for i in (0, 1):
                            sl = slice(4 * i, 4 * (i + 1))
                            nc.gpsimd.tensor_tensor(
                                y_sb[:, sl, h * U:(h + 1) * U], t[:, sl, :],
                                w0xn[:, sl, :], mybir.AluOpType.add,
                            )
                        # (last head's stats are emitted in phase 3,
                        # interleaved per-nt with the aggregates)
                    else:
                        znew = zpool.tile([P, NT, U], ZDT, tag="z")
                        for i in (0, 1):
                            sl = slice(4 * i, 4 * (i + 1))
                            nc.gpsimd.tensor_tensor(
                                znew[:, sl, :], t[:, sl, :], w0[:, sl, :],
                                mybir.AluOpType.add)
                        z_prev = znew
                    # deferred work in the post-mult slack: 2 of next head's
                    # masks (1 DVE + 1 Pool) and 2 tiny stats per hop
                    for _ in range(2):
                        if next_masks:
                            next_masks.pop(0)()
                    for _ in range(2):
                        if stats_prev:
                            stats_prev.pop(0)()

            # Schedule: q/k for it=0 first, then head 0's scores so exp(h0)
            # (8.3us of ACT) overlaps the rest of phase 1 on PE; then the
            # per-head pipeline (scores h+1 issued before hops h).
            qk_proj(0)
            e0, _, _ = scores_exp_mask(0, defer=False, pool_masks=True)
            for it in range(1, KT):
                qk_proj(it)
            v_proj()
            e_prev = e0
            for h in range(1, NHEADS):
                e_next, s_next, m_next = scores_exp_mask(h)
                hops(h - 1, e_prev, s_next, m_next)
                e_prev = e_next
            hops(NHEADS - 1, e_prev)

            # ---- phase 3: LayerNorm (residual already folded into y) ----
            # All stats/sqrt/recip first (tiny), then TSPs on two engines,
            # then DMAs on three queues — keeps sqrts from queuing behind
            # 790ns out-DMA transfers on the ACT queue.
            out_r = out_d[:, :].rearrange("(t p) h -> p t h", p=P)
            dma_engs = [nc.sync, nc.scalar, nc.gpsimd]
            st2s, rstds = [], []
            hl = NHEADS - 1
            for nt in range(NT):
                nc.vector.bn_stats(
                    st6_sb[:, nt, hl, :], y_sb[:, nt, hl * U:(hl + 1) * U],
                )
                st2 = spool.tile([P, 2], F32, tag="st2", name=f"st2_{nt}")
                nc.vector.bn_aggr(st2[:], st6_sb[:, nt, :, :])
                sd = spool.tile([P, 1], F32, tag="sd")
                nc.scalar.activation(
                    sd[:], st2[:, 1:2], mybir.ActivationFunctionType.Sqrt,
                    bias=eps_sb[:, :],
                )
                rstd = spool.tile([P, 1], F32, tag="rstd", name=f"rstd_{nt}")
                nc.vector.reciprocal(rstd[:], sd[:])
                st2s.append(st2)
                rstds.append(rstd)
            for nt in range(NT):
                yt = y_sb[:, nt, :]
                ybt = ybf_sb[:, nt, :]
                tsp_eng = nc.gpsimd if nt % 2 == 0 else nc.vector
                if apply_affine:
                    tsp_eng.tensor_scalar(
                        yt, yt, st2s[nt][:, 0:1], rstds[nt][:],
                        mybir.AluOpType.subtract, mybir.AluOpType.mult,
                    )
                    nc.vector.tensor_tensor(yt, yt, gam_sb[:, :], mybir.AluOpType.mult)
                    nc.vector.tensor_tensor(ybt, yt, bet_sb[:, :], mybir.AluOpType.add)
                else:
                    tsp_eng.tensor_scalar(
                        ybt, yt, st2s[nt][:, 0:1], rstds[nt][:],
                        mybir.AluOpType.subtract, mybir.AluOpType.mult,
                    )
            # Pool's DMAs go last so they don't block its TSPs (queue FIFO)
            dma_map = [0, 1, 0, 1, 2, 0, 1, 2]
            for nt in range(NT):
                dma_engs[dma_map[nt]].dma_start(out_r[:, nt, :], ybf_sb[:, nt, :])

    nc.finalize()
    _BUILD_CACHE[key] = nc
    return nc


def make_in_maps(x, adj, Wq, bq, Wk, bk, Wv, bv, gamma, beta, edt_name, apply_affine):
    import ml_dtypes
    bf16 = ml_dtypes.bfloat16
    np_edt = np.float32 if edt_name == "float32" else bf16
    x = np.ascontiguousarray(np.asarray(x, np.float32))
    adj = np.asarray(adj)
    wqT = np.ascontiguousarray(np.asarray(Wq, np.float32).T.astype(bf16))
    wkT = np.ascontiguousarray(np.asarray(Wk, np.float32).T.astype(bf16))
    wvT01 = np.ascontiguousarray((ALPHA * np.asarray(Wv, np.float32)).T.astype(bf16))
    bq = np.asarray(bq, np.float32)
    bk = np.asarray(bk, np.float32)
    bv01 = (ALPHA * np.asarray(bv, np.float32)).astype(bf16)
    in_maps = []
    for b in range(B):
        m = {
            "xT": np.ascontiguousarray(x[b].T.astype(bf16)),
            "xn": x[b],
            "maskT": np.ascontiguousarray((adj[b] != 0).T.astype(np_edt)),
            "wqT": wqT, "wkT": wkT, "wvT01": wvT01,
            "bq": bq, "bk": bk, "bv01": bv01,
        }
        if apply_affine:
            m["gammar"] = np.ascontiguousarray(
                np.broadcast_to(np.asarray(gamma, np.float32), (P, H)))
            m["betar"] = np.ascontiguousarray(
                np.broadcast_to(np.asarray(beta, np.float32), (P, H)))
        in_maps.append(m)
    return in_maps


def kernel(x, adj, Wq, bq, Wk, bk, Wv, bv, gamma, beta, _trace=False):
    apply_affine = not (
        np.allclose(np.asarray(gamma), 1.0) and np.allclose(np.asarray(beta), 0.0)
    )
    nc = build_nc(E_DTYPE_NAME, apply_affine)
    in_maps = make_in_maps(
        x, adj, Wq, bq, Wk, bk, Wv, bv, gamma, beta, E_DTYPE_NAME, apply_affine
    )
    res = run_bass_kernel_spmd(nc, in_maps, list(range(B)), trace=_trace)
    out = np.stack([np.asarray(res.results[b]["out"]) for b in range(B)])
    if _trace:
        return out.astype(np.float32), res
    return out.astype(np.float32)

